# revision 8
# baseline (speedup 1.0000x reference)
"""AttentionBlock kernel for 8 TRN2 NeuronCores.

Reference computation (B=1, C=64, DQK=8, N=H*W=16384):
    q = Wq@xf + bq   [N,8]
    k = Wk@xf + bk   [N,8]
    v = Wv@xf + bv   [N,64]
    attn = softmax(q k^T)  (no scaling)
    out = gamma * (attn @ v)^T + x

Sharding: sequence-parallel over the N attention rows (2048 rows per
core); every core computes full K/V from the full x (tiny compute).
No collectives needed.

Per-core pipeline (all matmul dtypes bf16, accumulation fp32 in PSUM):
  prologue: K^T [8,N], q^T [8,2048], V' [N,65] (ones col -> denominator)
  m-loop (128 chunks x 2 n-stripes of 1024):
    S tile [128,1024] = K_j^T x q^T        (TensorE, PSUM)
    P = exp(S)                              (ScalarE, PSUM->SBUF bf16)
    num[65,1024] += V'_j^T x P              (TensorE, PSUM accumulate)
  epilogue: r = 1/num[64]; out = gamma*num[0:64]*r + xq; DMA out
"""

import numpy as np
import ml_dtypes

C = 64
DQK = 8
N_FULL = 16384
NCORES = 8


def build_bass(n=N_FULL, n_local=N_FULL // NCORES, w=1024, sw=512):
    """Build the single-core Bass program (same program runs SPMD on all
    cores; per-core behavior differs only through the xq input slice)."""
    import concourse.bass as bass
    import concourse.mybir as mybir
    import concourse.tile as tile
    from concourse import bacc
    from concourse.bass import ts

    FP32 = mybir.dt.float32
    BF16 = mybir.dt.bfloat16
    PSUM = bass.MemorySpace.PSUM

    chunks = n // 128          # m chunks
    stripes = n_local // w     # n stripes
    halves = w // sw           # sub-tiles per stripe (<=512 fp32 psum bank)
    vg = sw // C               # V chunks packed per misc psum bank
    vgroups = chunks // vg
    assert n % 128 == 0 and n_local % w == 0 and w % sw == 0
    assert sw % C == 0 and chunks % vg == 0

    nc = bacc.Bacc()

    x_ext = nc.declare_dram_parameter("x", [C, n], BF16, isOutput=False)
    xq_ext = nc.declare_dram_parameter("xq", [C, n_local], FP32, isOutput=False)
    xqb_ext = nc.declare_dram_parameter("xqb", [C, n_local], BF16, isOutput=False)
    wqt_ext = nc.declare_dram_parameter("wqt", [C + 1, DQK], BF16, isOutput=False)
    wkt_ext = nc.declare_dram_parameter("wkt", [C + 1, DQK], BF16, isOutput=False)
    wvt_ext = nc.declare_dram_parameter("wvt", [C + 1, C], BF16, isOutput=False)
    g_ext = nc.declare_dram_parameter("gamma", [1, 1], FP32, isOutput=False)
    out_ext = nc.declare_dram_parameter("out", [C, n_local], FP32, isOutput=True)

    with tile.TileContext(nc) as tc:
        with (
            tc.tile_pool(name="const", bufs=1) as const_pool,
            tc.tile_pool(name="xbuf", bufs=1) as xbuf_pool,
            tc.tile_pool(name="kv", bufs=1) as kv_pool,
            tc.tile_pool(name="pbuf", bufs=3) as p_pool,
            tc.tile_pool(name="ep", bufs=2) as ep_pool,
            tc.tile_pool(name="s_ps", bufs=2, space=PSUM) as s_ps,
            tc.tile_pool(name="num_ps", bufs=1, space=PSUM) as num_ps,
            tc.tile_pool(name="misc_ps", bufs=2, space=PSUM) as misc_ps,
        ):
            # ---- constants / inputs to SBUF ----
            wqt_sb = const_pool.tile([C + 1, DQK], BF16, name="wqt_sb")
            wkt_sb = const_pool.tile([C + 1, DQK], BF16, name="wkt_sb")
            wvt_sb = const_pool.tile([C + 1, C], BF16, name="wvt_sb")
            g_sb = const_pool.tile([1, 1], FP32, name="g_sb")
            ones_sb = const_pool.tile([1, C], FP32, name="ones_sb")
            zbias = const_pool.tile([128, 1], FP32, name="zbias")
            nc.sync.dma_start(wqt_sb[:], wqt_ext[:])
            nc.sync.dma_start(wkt_sb[:], wkt_ext[:])
            nc.sync.dma_start(wvt_sb[:], wvt_ext[:])
            nc.sync.dma_start(g_sb[:], g_ext[:])
            nc.vector.memset(ones_sb[:], 1.0)
            nc.vector.memset(zbias[:], 0.0)

            x_sb = xbuf_pool.tile([C + 1, n], BF16, name="x_sb")
            xq_sb = xbuf_pool.tile([C, n_local], FP32, name="xq_sb")
            xqb_sb = xbuf_pool.tile([C + 1, n_local], BF16, name="xqb_sb")
            nc.vector.memset(x_sb[C : C + 1, :], 1.0)
            nc.vector.memset(xqb_sb[C : C + 1, :], 1.0)
            xdma = max(n // 8, 512)
            for i in range(n // xdma):
                nc.sync.dma_start(
                    x_sb[0:C, i * xdma : (i + 1) * xdma],
                    x_ext[:, i * xdma : (i + 1) * xdma],
                )
            nc.sync.dma_start(xq_sb[:], xq_ext[:])
            nc.sync.dma_start(xqb_sb[0:C, :], xqb_ext[:])

            # ---- prologue: qT, KT, V' ----
            kT_sb = kv_pool.tile([DQK, n], BF16, name="kT_sb")
            qT_sb = kv_pool.tile([DQK, n_local], BF16, name="qT_sb")
            vp_sb = kv_pool.tile([128, chunks * (C + 1)], BF16, name="vp_sb")
            vp_view = vp_sb.rearrange("p (j c) -> p j c", c=C + 1)
            # ones column of V' = softmax denominator accumulator
            nc.vector.memset(vp_view[:, :, C : C + 1], 1.0)

            for t in range(n_local // sw):
                q_ps = misc_ps.tile([DQK, sw], FP32, name="q_ps", tag="misc")
                nc.tensor.matmul(
                    q_ps[:], wqt_sb[:], xqb_sb[:, ts(t, sw)], start=True, stop=True
                )
                nc.vector.tensor_copy(qT_sb[:, ts(t, sw)], q_ps[:])

            for t in range(n // sw):
                k_ps = misc_ps.tile([DQK, sw], FP32, name="k_ps", tag="misc")
                nc.tensor.matmul(
                    k_ps[:], wkt_sb[:], x_sb[:, ts(t, sw)], start=True, stop=True
                )
                nc.vector.tensor_copy(kT_sb[:, ts(t, sw)], k_ps[:])

            for g in range(vgroups):
                v_ps = misc_ps.tile([128, vg * C], FP32, name="v_ps", tag="misc")
                for jj in range(vg):
                    j = g * vg + jj
                    nc.tensor.matmul(
                        v_ps[:, ts(jj, C)],
                        x_sb[:, ts(j, 128)],
                        wvt_sb[:],
                        start=True,
                        stop=True,
                    )
                nc.vector.tensor_copy(
                    vp_view[:, g * vg : (g + 1) * vg, 0:C],
                    v_ps[:].rearrange("p (j c) -> p j c", c=C),
                )

            # ---- main attention loop ----
            Exp = mybir.ActivationFunctionType.Exp
            for s in range(stripes):
                num_t = num_ps.tile([C + 1, w], FP32, name="num_t", tag="num")
                for j in range(chunks):
                    s_t = s_ps.tile([128, w], FP32, name="s_t", tag="s")
                    for h in range(halves):
                        nc.tensor.matmul(
                            s_t[:, ts(h, sw)],
                            kT_sb[:, ts(j, 128)],
                            qT_sb[:, s * w + h * sw : s * w + (h + 1) * sw],
                            start=True,
                            stop=True,
                        )
                    p_t = p_pool.tile([128, w], BF16, name="p_t", tag="p")
                    nc.scalar.activation(p_t[:], s_t[:], Exp, bias=zbias[:])
                    for h in range(halves):
                        nc.tensor.matmul(
                            num_t[:, ts(h, sw)],
                            vp_view[:, j, :],
                            p_t[:, ts(h, sw)],
                            start=(j == 0),
                            stop=(j == chunks - 1),
                        )

                # ---- epilogue for this stripe ----
                d_sb = ep_pool.tile([1, w], FP32, name="d_sb", tag="d")
                nc.vector.tensor_copy(d_sb[:], num_t[C : C + 1, :])
                r_sb = ep_pool.tile([1, w], FP32, name="r_sb", tag="r")
                nc.vector.reciprocal(r_sb[:], d_sb[:])
                rg_sb = ep_pool.tile([1, w], FP32, name="rg_sb", tag="rg")
                nc.vector.tensor_scalar_mul(rg_sb[:], r_sb[:], g_sb[:])
                for h in range(halves):
                    bc_ps = misc_ps.tile([C, sw], FP32, name="bc_ps", tag="misc")
                    nc.tensor.matmul(
                        bc_ps[:], ones_sb[:], rg_sb[:, ts(h, sw)], start=True, stop=True
                    )
                    bc_sb = ep_pool.tile([C, sw], FP32, name="bc_sb", tag="bc")
                    nc.vector.tensor_copy(bc_sb[:], bc_ps[:])
                    t_sb = ep_pool.tile([C, sw], FP32, name="t_sb", tag="t")
                    nc.vector.tensor_mul(t_sb[:], num_t[0:C, ts(h, sw)], bc_sb[:])
                    o_sb = ep_pool.tile([C, sw], FP32, name="o_sb", tag="o")
                    nc.vector.tensor_add(
                        o_sb[:],
                        t_sb[:],
                        xq_sb[0:C, s * w + h * sw : s * w + (h + 1) * sw],
                    )
                    nc.sync.dma_start(
                        out_ext[:, s * w + h * sw : s * w + (h + 1) * sw], o_sb[:]
                    )

    nc.finalize()
    return nc


def _make_in_maps(x, Wq, bq, Wk, bk, Wv, bv, gamma, n, n_local):
    bf16 = ml_dtypes.bfloat16
    xf = np.asarray(x, dtype=np.float32).reshape(C, n)
    wqt = np.concatenate([np.asarray(Wq).T, np.asarray(bq)[None, :]], 0).astype(bf16)
    wkt = np.concatenate([np.asarray(Wk).T, np.asarray(bk)[None, :]], 0).astype(bf16)
    wvt = np.concatenate([np.asarray(Wv).T, np.asarray(bv)[None, :]], 0).astype(bf16)
    x_bf = xf.astype(bf16)
    g = np.asarray(gamma, dtype=np.float32).reshape(1, 1)
    in_maps = []
    for i in range(NCORES):
        xq = np.ascontiguousarray(xf[:, i * n_local : (i + 1) * n_local])
        in_maps.append(
            {
                "x": x_bf,
                "xq": xq,
                "xqb": xq.astype(bf16),
                "wqt": wqt,
                "wkt": wkt,
                "wvt": wvt,
                "gamma": g,
            }
        )
    return in_maps


_CACHE = {}


def _get_nc():
    if "nc" not in _CACHE:
        _CACHE["nc"] = build_bass()
    return _CACHE["nc"]


def kernel(x, Wq, bq, Wk, bk, Wv, bv, gamma):
    from concourse.bass_utils import run_bass_kernel_spmd

    n = N_FULL
    n_local = n // NCORES
    nc = _get_nc()
    in_maps = _make_in_maps(x, Wq, bq, Wk, bk, Wv, bv, gamma, n, n_local)
    res = run_bass_kernel_spmd(nc, in_maps, core_ids=list(range(NCORES)))
    out = np.concatenate(
        [np.asarray(res.results[i]["out"]) for i in range(NCORES)], axis=1
    )
    return out.reshape(1, C, 128, 128).astype(np.float32)
